# revision 1
# baseline (speedup 1.0000x reference)
"""Trainium2 Bass kernel for a 2-layer LSTM decoder (nn_Decoder).

Strategy: tensor-parallel over the hidden dimension across 8 NeuronCores.
Each core owns a 128-wide slice of H=1024 for both LSTM layers (its 512 of
the 4096 gate rows), and replicates the final fc layer so the autoregressive
input y needs no exchange.  The only cross-core traffic is an allgather of
each layer's hidden-state slice (128x64 fp32 = 32KB) per step, done with
direct SBUF->SBUF remote DMA (no HBM bounce, no ncfw collective floor).

Layouts are feature-on-partition / batch-on-free ("transposed") everywhere.
LSTM gate matmuls run moving-weight orientation: out[batch(64), gates(512)]
= lhsT(activations.T).T @ rhs(W.T) in bf16 (1 cycle/row at N=512), fp32 PSUM
accumulation; gates are then PE-transposed to [hid(128), batch(64)] so
sigmoid/tanh biases ride free on the scalar engine's per-partition bias
operand and the new h slice lands directly in allgather layout.  The fc
matmul runs weight-stationary and produces y.T in PSUM directly.  Cell
state c stays fp32 on its own core slice.

Each exchange is ONE remote_dma_broadcast to the 7 peers (XOR-relative
routing; empirically the D2D lanes (delta bit 2 set) deliver to delta^2 so
those rdests are pre-compensated).  The destination slot is sender-pid *
64 columns via a register access pattern, so every core runs an identical
program.  Descriptor generation is pre-staged one step ahead on the Q7;
only trigger_dma sits on the critical path.

Flow control: h buffers are double-buffered by step parity; the LSTM's own
data-dependency chain guarantees a sender cannot run 2 steps ahead of any
receiver, so no credit messages are needed.
"""

import sys

sys.path.insert(0, "/opt/trn_rl_repo")

import numpy as np

B = 64
H = 1024
OUT = 512
N_CORES = 8
HPC = H // N_CORES  # 128 hidden units per core
SOS_VALUE = -2.0

_CACHE = {}


def _build(seq, exchanges=True, outdma=True, rswaits=True, one_bcast=True):
    from concourse import bacc, bass, mybir

    dt = mybir.dt
    f32 = dt.float32
    bf16 = dt.bfloat16
    AF = mybir.ActivationFunctionType
    ALU = mybir.AluOpType

    nc = bacc.Bacc("TRN2", target_bir_lowering=False, debug=False,
                   num_devices=N_CORES)

    # ---- DRAM I/O (per-core shards prepared on host) ----
    w0_d = nc.dram_tensor("w0", [12 * 128, 512], bf16, kind="ExternalInput")
    w1_d = nc.dram_tensor("w1", [16 * 128, 512], bf16, kind="ExternalInput")
    wfc_d = nc.dram_tensor("wfc", [8 * 128, 512], bf16, kind="ExternalInput")
    b0_d = nc.dram_tensor("b0", [128, 4], f32, kind="ExternalInput")
    b1_d = nc.dram_tensor("b1", [128, 4], f32, kind="ExternalInput")
    bfc_d = nc.dram_tensor("bfc", [128, 4], f32, kind="ExternalInput")
    h0i_d = nc.dram_tensor("h0i", [1024, 64], bf16, kind="ExternalInput")
    h1i_d = nc.dram_tensor("h1i", [1024, 64], bf16, kind="ExternalInput")
    c0i_d = nc.dram_tensor("c0i", [128, 64], f32, kind="ExternalInput")
    c1i_d = nc.dram_tensor("c1i", [128, 64], f32, kind="ExternalInput")
    yi_d = nc.dram_tensor("yi", [512, 64], bf16, kind="ExternalInput")
    id_d = nc.dram_tensor("ident", [128, 128], f32, kind="ExternalInput")
    out_d = nc.dram_tensor("out", [seq, 128, 256], bf16, kind="ExternalOutput")

    # ---- SBUF ----
    w0 = nc.alloc_sbuf_tensor("w0s", [128, 12 * 512], bf16)
    w1 = nc.alloc_sbuf_tensor("w1s", [128, 16 * 512], bf16)
    wfc = nc.alloc_sbuf_tensor("wfcs", [128, 8 * 512], bf16)
    b0 = nc.alloc_sbuf_tensor("b0s", [128, 4], f32)
    b1 = nc.alloc_sbuf_tensor("b1s", [128, 4], f32)
    bfc = nc.alloc_sbuf_tensor("bfcs", [128, 4], f32)
    ident = nc.alloc_sbuf_tensor("idents", [128, 128], f32)
    h0T = [nc.alloc_sbuf_tensor(f"h0T{p}", [128, 512], bf16) for p in range(2)]
    h1T = [nc.alloc_sbuf_tensor(f"h1T{p}", [128, 512], bf16) for p in range(2)]
    xT = [nc.alloc_sbuf_tensor(f"xT{p}", [128, 256], bf16) for p in range(2)]
    cT = [nc.alloc_sbuf_tensor(f"cT{l}", [128, 64], f32) for l in range(2)]
    g0sb = nc.alloc_sbuf_tensor("g0sb", [64, 512], f32)
    g1sb = nc.alloc_sbuf_tensor("g1sb", [64, 512], f32)
    gl = [[nc.alloc_sbuf_tensor(f"g{l}_{n}", [128, 64], f32)
           for n in ("i", "f", "g", "o", "th", "t1", "t2")] for l in range(2)]

    # ---- PSUM ----
    ps_g0 = nc.alloc_psum_tensor("ps_g0", [64, 512], f32)
    ps_g1 = nc.alloc_psum_tensor("ps_g1", [64, 512], f32)
    ps_t0 = nc.alloc_psum_tensor("ps_t0", [128, 256], f32)
    ps_t1 = nc.alloc_psum_tensor("ps_t1", [128, 256], f32)
    ps_ty = nc.alloc_psum_tensor("ps_ty", [128, 256], f32)

    # ---- semaphores ----
    S = lambda n: nc.alloc_semaphore(n)
    init = S("init")
    pe_g0, pe_t0 = S("pe_g0"), S("pe_t0")
    pe_g1, pe_t1 = S("pe_g1"), S("pe_t1")
    pe_ty = S("pe_ty")
    act_g0, act_th0 = S("act_g0"), S("act_th0")
    act_g1, act_th1 = S("act_g1"), S("act_th1")
    act_y = S("act_y")
    dv_g0, dv_g1 = S("dv_g0"), S("dv_g1")
    dv_c0, dv_c1 = S("dv_c0"), S("dv_c1")
    dv_h0, dv_h1 = S("dv_h0"), S("dv_h1")
    prep = S("prep")
    ls0, ls1 = S("ls0"), S("ls1")
    rs_h0 = [S(f"rs_h0_{p}") for p in range(2)]
    rs_h1 = [S(f"rs_h1_{p}") for p in range(2)]
    dsem = S("dsem")

    N_INIT_DMA = 12
    INIT_V = 16 * N_INIT_DMA

    LS_PER = 16 if one_bcast else 112

    def fills(t):
        # number of exchange rounds into buffer t%2 after step t's exchange
        return t // 2 + 1

    def wtile(sb, k):
        return sb.ap()[:, 512 * k:512 * (k + 1)]

    def htile(sb, k):
        return sb.ap()[:, 64 * k:64 * (k + 1)]

    with nc.Block() as block:

        @block.sync
        def _(eng):
            eng.dma_start(
                w0.ap().rearrange("p (t n) -> p t n", t=12),
                w0_d.ap().rearrange("(t p) n -> p t n", p=128)).then_inc(init, 16)
            eng.dma_start(
                w1.ap().rearrange("p (t n) -> p t n", t=16),
                w1_d.ap().rearrange("(t p) n -> p t n", p=128)).then_inc(init, 16)
            eng.dma_start(
                wfc.ap().rearrange("p (t n) -> p t n", t=8),
                wfc_d.ap().rearrange("(t p) n -> p t n", p=128)).then_inc(init, 16)
            eng.dma_start(b0.ap(), b0_d.ap()).then_inc(init, 16)
            eng.dma_start(b1.ap(), b1_d.ap()).then_inc(init, 16)
            eng.dma_start(bfc.ap(), bfc_d.ap()).then_inc(init, 16)
            eng.dma_start(
                h0T[1].ap().rearrange("p (t n) -> p t n", t=8),
                h0i_d.ap().rearrange("(t p) n -> p t n", p=128)).then_inc(init, 16)
            eng.dma_start(
                h1T[1].ap().rearrange("p (t n) -> p t n", t=8),
                h1i_d.ap().rearrange("(t p) n -> p t n", p=128)).then_inc(init, 16)
            eng.dma_start(cT[0].ap(), c0i_d.ap()).then_inc(init, 16)
            eng.dma_start(cT[1].ap(), c1i_d.ap()).then_inc(init, 16)
            eng.dma_start(
                xT[1].ap().rearrange("p (t n) -> p t n", t=4),
                yi_d.ap().rearrange("(t p) n -> p t n", p=128)).then_inc(init, 16)
            eng.dma_start(ident.ap(), id_d.ap()).then_inc(init, 16)
            for t in range(seq if outdma else 0):
                eng.wait_ge(act_y, t + 1)
                eng.dma_start(
                    out_d.ap()[t], xT[t % 2].ap()).then_inc(dsem, 16)

        @block.tensor
        def _(eng):
            eng.wait_ge(init, INIT_V)
            # prologue: L0 hh-part for t=0 (reads initial h0 in buf 1)
            for k in range(8):
                nc.tensor.matmul(ps_g0.ap(), htile(h0T[1], k), wtile(w0, 4 + k),
                                 start=(k == 0), stop=False)
            for t in range(seq):
                p, q = t % 2, (t + 1) % 2
                # ---- layer 0 gates: close the group with the x-part ----
                if t >= 1:
                    eng.wait_ge(act_y, t)        # x = y(t-1) ready in xT[q]
                for k in range(4):
                    mm = nc.tensor.matmul(ps_g0.ap(), htile(xT[q], k),
                                          wtile(w0, k),
                                          start=False, stop=(k == 3))
                mm.then_inc(pe_g0, 1)
                # early L1-hh matmuls overlap the DVE gate copy
                if t >= 1:
                    eng.wait_ge(dv_g1, t)
                    eng.wait_ge(dv_h1, t)
                    if exchanges and rswaits:
                        eng.wait_ge(rs_h1[q], 14 * fills(t - 1))
                for k in range(3):
                    nc.tensor.matmul(ps_g1.ap(), htile(h1T[q], k),
                                     wtile(w1, 8 + k),
                                     start=(k == 0), stop=False)
                # ---- transpose gates0 to [128, 4*64] ----
                eng.wait_ge(dv_g0, t + 1)        # g0sb written by DVE
                if t >= 1:
                    eng.wait_ge(act_g0, t)       # ps_t0 consumed by ACT
                for j in range(4):
                    mm = nc.tensor.matmul(ps_t0.ap()[:, 64 * j:64 * (j + 1)],
                                          g0sb.ap()[:, 128 * j:128 * (j + 1)],
                                          ident.ap()[:64, :64],
                                          is_transpose=True, start=True,
                                          stop=True)
                mm.then_inc(pe_t0, 1)
                # ---- layer 1 gates: finish hh-part, then fresh-h0 ih-part ----
                for k in range(3, 8):
                    nc.tensor.matmul(ps_g1.ap(), htile(h1T[q], k),
                                     wtile(w1, 8 + k),
                                     start=False, stop=False)
                eng.wait_ge(dv_h0, t + 1)        # own h0(t) slice
                if exchanges and rswaits:
                    eng.wait_ge(rs_h0[p], 14 * fills(t))  # peers' h0(t)
                for k in range(8):
                    mm = nc.tensor.matmul(ps_g1.ap(), htile(h0T[p], k),
                                          wtile(w1, k),
                                          start=False, stop=(k == 7))
                mm.then_inc(pe_g1, 1)
                # ---- transpose gates1 ----
                eng.wait_ge(dv_g1, t + 1)
                if t >= 1:
                    eng.wait_ge(act_g1, t)
                for j in range(4):
                    mm = nc.tensor.matmul(ps_t1.ap()[:, 64 * j:64 * (j + 1)],
                                          g1sb.ap()[:, 128 * j:128 * (j + 1)],
                                          ident.ap()[:64, :64],
                                          is_transpose=True, start=True,
                                          stop=True)
                mm.then_inc(pe_t1, 1)
                # ---- L0 hh-part for step t+1 (fills the h1-exchange window;
                # h0(t) already gathered, ps_g0 drained once dv_g0 hits t+1) ----
                if t + 1 < seq:
                    eng.wait_ge(dv_g0, t + 1)
                    eng.wait_ge(dv_h0, t + 1)
                    if exchanges and rswaits:
                        eng.wait_ge(rs_h0[p], 14 * fills(t))
                    for k in range(8):
                        nc.tensor.matmul(ps_g0.ap(), htile(h0T[p], k),
                                         wtile(w0, 4 + k),
                                         start=(k == 0), stop=False)
                # ---- fc (replicated, weight-stationary): y.T into ps_ty ----
                eng.wait_ge(dv_h1, t + 1)
                if exchanges and rswaits:
                    eng.wait_ge(rs_h1[p], 14 * fills(t))
                if t >= 1:
                    eng.wait_ge(act_y, t)        # ps_ty consumed by ACT(t-1)
                for m in range(4):
                    for k in range(8):
                        mm = nc.tensor.matmul(
                            ps_ty.ap()[:, 64 * m:64 * (m + 1)],
                            wfc.ap()[:, 512 * k + 128 * m:512 * k + 128 * (m + 1)],
                            htile(h1T[p], k),
                            start=(k == 0), stop=(k == 7))
                mm.then_inc(pe_ty, 1)

        @block.scalar
        def _(eng):
            eng.wait_ge(init, INIT_V)
            for t in range(seq):
                p = t % 2
                for l, (ps_t, gsem, thsem, csem, bias) in enumerate(
                        ((ps_t0, act_g0, act_th0, dv_c0, b0),
                         (ps_t1, act_g1, act_th1, dv_c1, b1))):
                    eng.wait_ge((pe_t0, pe_t1)[l], t + 1)
                    i_t, f_t, g_t, o_t, th_t = [x.ap() for x in gl[l][:5]]
                    src = ps_t.ap()
                    a = nc.scalar.activation(i_t, src[:, 0:64], AF.Sigmoid,
                                             bias=bias.ap()[:, 0:1])
                    a = nc.scalar.activation(f_t, src[:, 64:128], AF.Sigmoid,
                                             bias=bias.ap()[:, 1:2])
                    a = nc.scalar.activation(g_t, src[:, 128:192], AF.Tanh,
                                             bias=bias.ap()[:, 2:3])
                    a = nc.scalar.activation(o_t, src[:, 192:256], AF.Sigmoid,
                                             bias=bias.ap()[:, 3:4])
                    a.then_inc(gsem, 1)
                    eng.wait_ge(csem, t + 1)
                    nc.scalar.activation(th_t, cT[l].ap(),
                                         AF.Tanh).then_inc(thsem, 1)
                # fc relu -> xT[p]
                eng.wait_ge(pe_ty, t + 1)
                if t >= 2 and outdma:
                    eng.wait_ge(dsem, 16 * (t - 1))   # out-DMA(t-2) done
                for j in range(4):
                    a = nc.scalar.activation(xT[p].ap()[:, 64 * j:64 * (j + 1)],
                                             ps_ty.ap()[:, 64 * j:64 * (j + 1)],
                                             AF.Relu, bias=bfc.ap()[:, j:j + 1])
                a.then_inc(act_y, 1)

        @block.vector
        def _(eng):
            eng.wait_ge(init, INIT_V)
            dv_off = eng.partition_id() * 64 if one_bcast else None
            for t in range(seq):
                p = t % 2
                # layer 0
                eng.wait_ge(pe_g0, t + 1)
                nc.vector.tensor_copy(g0sb.ap(), ps_g0.ap()).then_inc(dv_g0, 1)
                eng.wait_ge(act_g0, t + 1)
                i_t, f_t, g_t, o_t, th_t, t1, t2 = [x.ap() for x in gl[0]]
                nc.vector.tensor_tensor(t1, f_t, cT[0].ap(), ALU.mult)
                nc.vector.tensor_tensor(t2, i_t, g_t, ALU.mult)
                if t >= 1:
                    eng.wait_ge(act_th0, t)      # tanh(c(t-1)) read done
                nc.vector.tensor_tensor(cT[0].ap(), t1, t2,
                                        ALU.add).then_inc(dv_c0, 1)
                eng.wait_ge(act_th0, t + 1)
                if t >= 2 and exchanges:
                    eng.wait_ge(ls0, LS_PER * (t - 1))  # sends from buf p drained
                h0slot = (h0T[p].ap()[:, bass.ds(dv_off, 64)] if one_bcast
                          else h0T[p].ap()[:, 0:64])
                nc.vector.tensor_tensor(h0slot, o_t, th_t,
                                        ALU.mult).then_inc(dv_h0, 1)
                # layer 1
                eng.wait_ge(pe_g1, t + 1)
                nc.vector.tensor_copy(g1sb.ap(), ps_g1.ap()).then_inc(dv_g1, 1)
                eng.wait_ge(act_g1, t + 1)
                i_t, f_t, g_t, o_t, th_t, t1, t2 = [x.ap() for x in gl[1]]
                nc.vector.tensor_tensor(t1, f_t, cT[1].ap(), ALU.mult)
                nc.vector.tensor_tensor(t2, i_t, g_t, ALU.mult)
                if t >= 1:
                    eng.wait_ge(act_th1, t)
                nc.vector.tensor_tensor(cT[1].ap(), t1, t2,
                                        ALU.add).then_inc(dv_c1, 1)
                eng.wait_ge(act_th1, t + 1)
                if t >= 2 and exchanges:
                    eng.wait_ge(ls1, LS_PER * (t - 1))
                h1slot = (h1T[p].ap()[:, bass.ds(dv_off, 64)] if one_bcast
                          else h1T[p].ap()[:, 0:64])
                nc.vector.tensor_tensor(h1slot, o_t, th_t,
                                        ALU.mult).then_inc(dv_h1, 1)

        @block.gpsimd
        def _(eng):
            eng.wait_ge(init, INIT_V)
            if one_bcast and exchanges:
                gp_off = eng.partition_id() * 64
                rdests = [None] + [(0, d ^ 2) if d >= 4 else (0, d)
                                   for d in range(1, 8)]

                def stage(t):
                    p = t % 2
                    for buf, rsem, lsem in ((h0T[p], rs_h0[p], ls0),
                                            (h1T[p], rs_h1[p], ls1)):
                        slot = buf.ap()[:, bass.ds(gp_off, 64)]
                        eng.remote_dma_broadcast(
                            slot, slot, remote_sem=rsem, local_sem=lsem,
                            rdests=rdests).then_inc(prep, 1)

                stage(0)
                for t in range(seq):
                    eng.wait_ge(prep, 2 * t + 1)
                    eng.wait_ge(dv_h0, t + 1)
                    eng.trigger_dma(count=1)
                    eng.wait_ge(prep, 2 * t + 2)
                    eng.wait_ge(dv_h1, t + 1)
                    eng.trigger_dma(count=1)
                    if t + 1 < seq:
                        stage(t + 1)
            else:
                gp_off = eng.partition_id() * 64 if one_bcast else None
                nprep = 0
                for t in range(seq if exchanges else 0):
                    p = t % 2
                    for buf, hsem, rsem, lsem in ((h0T[p], dv_h0, rs_h0[p], ls0),
                                                  (h1T[p], dv_h1, rs_h1[p], ls1)):
                        eng.wait_ge(hsem, t + 1)
                        for d in range(1, 8):
                            rdests2 = [None] * 8
                            rdests2[d] = (0, d ^ 2) if d >= 4 else (0, d)
                            eng.remote_dma_broadcast(
                                buf.ap()[:, 64 * d:64 * (d + 1)],
                                buf.ap()[:, 0:64],
                                remote_sem=rsem, local_sem=lsem,
                                rdests=rdests2).then_inc(prep, 1)
                        nprep += 7
                        eng.wait_ge(prep, nprep)
                        eng.trigger_dma(count=7)

    nc.compile()
    return nc


def _prep_inputs(core, W_ih0, W_hh0, b_ih0, b_hh0, W_ih1, W_hh1, b_ih1, b_hh1,
                 W_fc, b_fc, h0, c0, rotate=False):
    c = core
    rows = np.concatenate([np.arange(g * H + c * HPC, g * H + (c + 1) * HPC)
                           for g in range(4)])
    if rotate:
        hperm = np.concatenate([np.arange((c ^ j) * HPC, ((c ^ j) + 1) * HPC)
                                for j in range(8)])
    else:
        hperm = np.arange(H)
    import ml_dtypes
    f = np.float32
    bf = ml_dtypes.bfloat16
    w0 = np.concatenate([W_ih0[rows].T, W_hh0[rows].T[hperm]], axis=0)
    w1 = np.concatenate([W_ih1[rows].T[hperm], W_hh1[rows].T[hperm]], axis=0)
    wfc = W_fc.T[hperm]
    return {
        "w0": np.ascontiguousarray(w0).astype(bf),
        "w1": np.ascontiguousarray(w1).astype(bf),
        "wfc": np.ascontiguousarray(wfc).astype(bf),
        "b0": np.ascontiguousarray((b_ih0 + b_hh0)[rows].reshape(4, HPC).T, f),
        "b1": np.ascontiguousarray((b_ih1 + b_hh1)[rows].reshape(4, HPC).T, f),
        "bfc": np.ascontiguousarray(b_fc.reshape(4, HPC).T, f),
        "h0i": np.ascontiguousarray(h0[0].T[hperm]).astype(bf),
        "h1i": np.ascontiguousarray(h0[1].T[hperm]).astype(bf),
        "c0i": np.ascontiguousarray(c0[0][:, c * HPC:(c + 1) * HPC].T, f),
        "c1i": np.ascontiguousarray(c0[1][:, c * HPC:(c + 1) * HPC].T, f),
        "yi": np.full((512, 64), SOS_VALUE, bf),
        "ident": np.eye(128, dtype=f),
    }


def run(seq, in_maps, trace=False, trace_kwargs=None):
    from concourse.bass_utils import run_bass_kernel_spmd

    key = int(seq)
    if key not in _CACHE:
        _CACHE[key] = _build(key)
    nc = _CACHE[key]
    kw = {}
    if trace:
        kw = dict(trace=True, trace_cores=[0], **(trace_kwargs or {}))
    return run_bass_kernel_spmd(nc, in_maps, core_ids=list(range(N_CORES)),
                                **kw)


def kernel(encoder_output=None, h0=None, c0=None, W_ih0=None, W_hh0=None,
           b_ih0=None, b_hh0=None, W_ih1=None, W_hh1=None, b_ih1=None,
           b_hh1=None, W_fc=None, b_fc=None, seq_length=256, _trace=False):
    seq = int(seq_length)
    args = (W_ih0, W_hh0, b_ih0, b_hh0, W_ih1, W_hh1, b_ih1, b_hh1, W_fc, b_fc,
            h0, c0)
    args = tuple(np.asarray(a, np.float32) for a in args)
    in_maps = [_prep_inputs(c, *args) for c in range(N_CORES)]
    res = run(seq, in_maps, trace=_trace)
    out = np.asarray(res.results[0]["out"]).astype(np.float32)
    y = out.reshape(seq, 128, 4, 64).transpose(3, 0, 2, 1).reshape(B, seq, OUT)
    if _trace:
        kernel._last_results = res
    return np.ascontiguousarray(y)



# revision 27
# speedup vs baseline: 9.6137x; 9.6137x over previous
"""Trainium2 Bass kernel for a 2-layer LSTM decoder (nn_Decoder).

Strategy: tensor-parallel over the hidden dimension across 8 NeuronCores.
Each core owns a 128-wide slice of H=1024 for both LSTM layers (its 512 of
the 4096 gate rows), and replicates the final fc layer so the autoregressive
input y needs no exchange.  The only cross-core traffic is an allgather of
each layer's hidden-state slice (128x64 fp32 = 32KB) per step, done with
direct SBUF->SBUF remote DMA (no HBM bounce, no ncfw collective floor).

Layouts are feature-on-partition / batch-on-free ("transposed") everywhere.
LSTM gate matmuls run moving-weight orientation: out[batch(64), gates(512)]
= lhsT(activations.T).T @ rhs(W.T) in bf16 (1 cycle/row at N=512), fp32 PSUM
accumulation; gates are then PE-transposed to [hid(128), batch(64)] so
sigmoid/tanh biases ride free on the scalar engine's per-partition bias
operand and the new h slice lands directly in allgather layout.  The fc
matmul runs weight-stationary and produces y.T in PSUM directly.  Cell
state c stays fp32 on its own core slice.

Each exchange is ONE remote_dma_broadcast to the 7 peers (XOR-relative
routing; empirically the D2D lanes (delta bit 2 set) deliver to delta^2 so
those rdests are pre-compensated).  The destination slot is sender-pid *
64 columns via a register access pattern, so every core runs an identical
program.  Descriptor generation is pre-staged one step ahead on the Q7;
only trigger_dma sits on the critical path.

A single 16KB broadcast frame takes ~12us from trigger to send-complete
(HW-measured with a sends-only microbench), so sends must be pipelined,
not serialized: h buffers are 4-deep (NBUF), the sender may run up to 3
sends ahead of the ls (send-complete) semaphore, and h0/h1 frames ride
separate SWDGE queues (0/1) so a step's two sends drain concurrently on
the SDMA engines.  The LSTM's own data-dependency chain (each step needs
all peers' h from the previous step) bounds sender/receiver skew well
below NBUF, so no credit messages are needed.
"""

import sys

sys.path.insert(0, "/opt/trn_rl_repo")

import numpy as np

B = 64
H = 1024
OUT = 512
N_CORES = 8
HPC = H // N_CORES  # 128 hidden units per core
SOS_VALUE = -2.0

_CACHE = {}


def _build(seq, exchanges=True, outdma=True, rswaits=True, one_bcast=True):
    from concourse import bacc, bass, mybir

    dt = mybir.dt
    f32 = dt.float32
    bf16 = dt.bfloat16
    AF = mybir.ActivationFunctionType
    ALU = mybir.AluOpType

    nc = bacc.Bacc("TRN2", target_bir_lowering=False, debug=False,
                   num_devices=N_CORES, num_swdge_queues=2)

    # ---- DRAM I/O (per-core shards prepared on host) ----
    w0_d = nc.dram_tensor("w0", [12 * 128, 512], bf16, kind="ExternalInput")
    w1_d = nc.dram_tensor("w1", [16 * 128, 512], bf16, kind="ExternalInput")
    wfc_d = nc.dram_tensor("wfc", [8 * 128, 512], bf16, kind="ExternalInput")
    b0_d = nc.dram_tensor("b0", [128, 4], f32, kind="ExternalInput")
    b1_d = nc.dram_tensor("b1", [128, 4], f32, kind="ExternalInput")
    bfc_d = nc.dram_tensor("bfc", [128, 4], f32, kind="ExternalInput")
    h0i_d = nc.dram_tensor("h0i", [1024, 64], bf16, kind="ExternalInput")
    h1i_d = nc.dram_tensor("h1i", [1024, 64], bf16, kind="ExternalInput")
    c0i_d = nc.dram_tensor("c0i", [128, 64], f32, kind="ExternalInput")
    c1i_d = nc.dram_tensor("c1i", [128, 64], f32, kind="ExternalInput")
    yi_d = nc.dram_tensor("yi", [512, 64], bf16, kind="ExternalInput")
    id_d = nc.dram_tensor("ident", [128, 128], f32, kind="ExternalInput")
    out_d = nc.dram_tensor("out", [seq, 128, 256], bf16, kind="ExternalOutput")

    # ---- SBUF ----
    w0 = nc.alloc_sbuf_tensor("w0s", [128, 12 * 512], bf16)
    w1 = nc.alloc_sbuf_tensor("w1s", [128, 16 * 512], bf16)
    wfc = nc.alloc_sbuf_tensor("wfcs", [128, 8 * 512], bf16)
    b0 = nc.alloc_sbuf_tensor("b0s", [128, 4], f32)
    b1 = nc.alloc_sbuf_tensor("b1s", [128, 4], f32)
    bfc = nc.alloc_sbuf_tensor("bfcs", [128, 4], f32)
    ident = nc.alloc_sbuf_tensor("idents", [128, 128], f32)
    NBUF = 4  # h-exchange depth: unchokes the SWDGE send chain
    h0T = [nc.alloc_sbuf_tensor(f"h0T{p}", [128, 512], bf16)
           for p in range(NBUF)]
    h1T = [nc.alloc_sbuf_tensor(f"h1T{p}", [128, 512], bf16)
           for p in range(NBUF)]
    xT = [nc.alloc_sbuf_tensor(f"xT{p}", [128, 256], bf16) for p in range(2)]
    cT = [nc.alloc_sbuf_tensor(f"cT{l}", [128, 64], f32) for l in range(2)]
    g0sb = nc.alloc_sbuf_tensor("g0sb", [64, 512], f32)
    g1sb = nc.alloc_sbuf_tensor("g1sb", [64, 512], f32)
    gl = [[nc.alloc_sbuf_tensor(f"g{l}_{n}", [128, 64], f32)
           for n in ("i", "f", "g", "o", "th", "t1", "t2")] for l in range(2)]

    # ---- PSUM ----
    ps_g0 = nc.alloc_psum_tensor("ps_g0", [64, 512], f32)
    ps_g1 = nc.alloc_psum_tensor("ps_g1", [64, 512], f32)
    ps_t0 = nc.alloc_psum_tensor("ps_t0", [128, 256], f32)
    ps_t1 = nc.alloc_psum_tensor("ps_t1", [128, 256], f32)
    ps_ty = nc.alloc_psum_tensor("ps_ty", [128, 256], f32)

    # ---- semaphores ----
    S = lambda n: nc.alloc_semaphore(n)
    init = S("init")
    pe_g0, pe_t0 = S("pe_g0"), S("pe_t0")
    pe_g1, pe_t1 = S("pe_g1"), S("pe_t1")
    pe_ty = S("pe_ty")
    act_g0, act_th0 = S("act_g0"), S("act_th0")
    act_g1, act_th1 = S("act_g1"), S("act_th1")
    act_y = S("act_y")
    dv_g0, dv_g1 = S("dv_g0"), S("dv_g1")
    dv_c0, dv_c1 = S("dv_c0"), S("dv_c1")
    dv_h0, dv_h1 = S("dv_h0"), S("dv_h1")
    prep = S("prep")
    ls0, ls1 = S("ls0"), S("ls1")
    rs_h0 = [S(f"rs_h0_{p}") for p in range(NBUF)]
    rs_h1 = [S(f"rs_h1_{p}") for p in range(NBUF)]
    dsem = S("dsem")

    N_INIT_DMA = 12
    INIT_V = 16 * N_INIT_DMA

    LS_PER = 16 if one_bcast else 112

    def fills(t):
        # number of exchange rounds into buffer t%NBUF after step t's exchange
        return t // NBUF + 1

    def wtile(sb, k):
        return sb.ap()[:, 512 * k:512 * (k + 1)]

    def htile(sb, k):
        return sb.ap()[:, 64 * k:64 * (k + 1)]

    with nc.Block() as block:

        @block.sync
        def _(eng):
            eng.dma_start(
                w0.ap().rearrange("p (t n) -> p t n", t=12),
                w0_d.ap().rearrange("(t p) n -> p t n", p=128)).then_inc(init, 16)
            eng.dma_start(
                w1.ap().rearrange("p (t n) -> p t n", t=16),
                w1_d.ap().rearrange("(t p) n -> p t n", p=128)).then_inc(init, 16)
            eng.dma_start(
                wfc.ap().rearrange("p (t n) -> p t n", t=8),
                wfc_d.ap().rearrange("(t p) n -> p t n", p=128)).then_inc(init, 16)
            eng.dma_start(b0.ap(), b0_d.ap()).then_inc(init, 16)
            eng.dma_start(b1.ap(), b1_d.ap()).then_inc(init, 16)
            eng.dma_start(bfc.ap(), bfc_d.ap()).then_inc(init, 16)
            eng.dma_start(
                h0T[NBUF - 1].ap().rearrange("p (t n) -> p t n", t=8),
                h0i_d.ap().rearrange("(t p) n -> p t n", p=128)).then_inc(init, 16)
            eng.dma_start(
                h1T[NBUF - 1].ap().rearrange("p (t n) -> p t n", t=8),
                h1i_d.ap().rearrange("(t p) n -> p t n", p=128)).then_inc(init, 16)
            eng.dma_start(cT[0].ap(), c0i_d.ap()).then_inc(init, 16)
            eng.dma_start(cT[1].ap(), c1i_d.ap()).then_inc(init, 16)
            eng.dma_start(
                xT[1].ap().rearrange("p (t n) -> p t n", t=4),
                yi_d.ap().rearrange("(t p) n -> p t n", p=128)).then_inc(init, 16)
            eng.dma_start(ident.ap(), id_d.ap()).then_inc(init, 16)
            for t in range(seq if outdma else 0):
                eng.wait_ge(act_y, t + 1)
                eng.dma_start(
                    out_d.ap()[t], xT[t % 2].ap()).then_inc(dsem, 16)

        @block.tensor
        def _(eng):
            eng.wait_ge(init, INIT_V)
            # prologue: L0 hh-part for t=0 (reads initial h0 in buf NBUF-1)
            for k in range(8):
                nc.tensor.matmul(ps_g0.ap(), htile(h0T[NBUF - 1], k),
                                 wtile(w0, 4 + k),
                                 start=(k == 0), stop=False)
            for t in range(seq):
                p, q = t % 2, (t + 1) % 2
                b, pb = t % NBUF, (t + NBUF - 1) % NBUF
                # ---- layer 0 gates: close the group with the x-part ----
                if t >= 1:
                    eng.wait_ge(act_y, t)        # x = y(t-1) ready in xT[q]
                for k in range(4):
                    mm = nc.tensor.matmul(ps_g0.ap(), htile(xT[q], k),
                                          wtile(w0, k),
                                          start=False, stop=(k == 3))
                mm.then_inc(pe_g0, 1)
                # early L1-hh matmuls overlap the DVE gate copy
                if t >= 1:
                    eng.wait_ge(dv_g1, t)
                    eng.wait_ge(dv_h1, t)
                    if exchanges and rswaits:
                        eng.wait_ge(rs_h1[pb], 14 * fills(t - 1))
                for k in range(3):
                    nc.tensor.matmul(ps_g1.ap(), htile(h1T[pb], k),
                                     wtile(w1, 8 + k),
                                     start=(k == 0), stop=False)
                # ---- transpose gates0 to [128, 4*64] ----
                eng.wait_ge(dv_g0, t + 1)        # g0sb written by DVE
                if t >= 1:
                    eng.wait_ge(act_g0, t)       # ps_t0 consumed by ACT
                for j in range(4):
                    mm = nc.tensor.matmul(ps_t0.ap()[:, 64 * j:64 * (j + 1)],
                                          g0sb.ap()[:, 128 * j:128 * (j + 1)],
                                          ident.ap()[:64, :64],
                                          is_transpose=True, start=True,
                                          stop=True)
                mm.then_inc(pe_t0, 1)
                # ---- layer 1 gates: finish hh-part, then fresh-h0 ih-part ----
                for k in range(3, 8):
                    nc.tensor.matmul(ps_g1.ap(), htile(h1T[pb], k),
                                     wtile(w1, 8 + k),
                                     start=False, stop=False)
                eng.wait_ge(dv_h0, t + 1)        # own h0(t) slice
                if exchanges and rswaits:
                    eng.wait_ge(rs_h0[b], 14 * fills(t))  # peers' h0(t)
                for k in range(8):
                    mm = nc.tensor.matmul(ps_g1.ap(), htile(h0T[b], k),
                                          wtile(w1, k),
                                          start=False, stop=(k == 7))
                mm.then_inc(pe_g1, 1)
                # ---- transpose gates1 ----
                eng.wait_ge(dv_g1, t + 1)
                if t >= 1:
                    eng.wait_ge(act_g1, t)
                for j in range(4):
                    mm = nc.tensor.matmul(ps_t1.ap()[:, 64 * j:64 * (j + 1)],
                                          g1sb.ap()[:, 128 * j:128 * (j + 1)],
                                          ident.ap()[:64, :64],
                                          is_transpose=True, start=True,
                                          stop=True)
                mm.then_inc(pe_t1, 1)
                # ---- L0 hh-part for step t+1 (fills the h1-exchange window;
                # h0(t) already gathered, ps_g0 drained once dv_g0 hits t+1) ----
                if t + 1 < seq:
                    eng.wait_ge(dv_g0, t + 1)
                    eng.wait_ge(dv_h0, t + 1)
                    if exchanges and rswaits:
                        eng.wait_ge(rs_h0[b], 14 * fills(t))
                    for k in range(8):
                        nc.tensor.matmul(ps_g0.ap(), htile(h0T[b], k),
                                         wtile(w0, 4 + k),
                                         start=(k == 0), stop=False)
                # ---- fc (replicated, weight-stationary): y.T into ps_ty ----
                eng.wait_ge(dv_h1, t + 1)
                if exchanges and rswaits:
                    eng.wait_ge(rs_h1[b], 14 * fills(t))
                if t >= 1:
                    eng.wait_ge(act_y, t)        # ps_ty consumed by ACT(t-1)
                for m in range(4):
                    for k in range(8):
                        mm = nc.tensor.matmul(
                            ps_ty.ap()[:, 64 * m:64 * (m + 1)],
                            wfc.ap()[:, 512 * k + 128 * m:512 * k + 128 * (m + 1)],
                            htile(h1T[b], k),
                            start=(k == 0), stop=(k == 7))
                mm.then_inc(pe_ty, 1)

        @block.scalar
        def _(eng):
            eng.wait_ge(init, INIT_V)
            for t in range(seq):
                p = t % 2
                for l, (ps_t, gsem, thsem, csem, bias) in enumerate(
                        ((ps_t0, act_g0, act_th0, dv_c0, b0),
                         (ps_t1, act_g1, act_th1, dv_c1, b1))):
                    eng.wait_ge((pe_t0, pe_t1)[l], t + 1)
                    i_t, f_t, g_t, o_t, th_t = [x.ap() for x in gl[l][:5]]
                    src = ps_t.ap()
                    a = nc.scalar.activation(i_t, src[:, 0:64], AF.Sigmoid,
                                             bias=bias.ap()[:, 0:1])
                    a = nc.scalar.activation(f_t, src[:, 64:128], AF.Sigmoid,
                                             bias=bias.ap()[:, 1:2])
                    a = nc.scalar.activation(g_t, src[:, 128:192], AF.Tanh,
                                             bias=bias.ap()[:, 2:3])
                    a = nc.scalar.activation(o_t, src[:, 192:256], AF.Sigmoid,
                                             bias=bias.ap()[:, 3:4])
                    a.then_inc(gsem, 1)
                    eng.wait_ge(csem, t + 1)
                    nc.scalar.activation(th_t, cT[l].ap(),
                                         AF.Tanh).then_inc(thsem, 1)
                # fc relu -> xT[p]
                eng.wait_ge(pe_ty, t + 1)
                if t >= 2 and outdma:
                    eng.wait_ge(dsem, 16 * (t - 1))   # out-DMA(t-2) done
                for j in range(4):
                    a = nc.scalar.activation(xT[p].ap()[:, 64 * j:64 * (j + 1)],
                                             ps_ty.ap()[:, 64 * j:64 * (j + 1)],
                                             AF.Relu, bias=bfc.ap()[:, j:j + 1])
                a.then_inc(act_y, 1)

        @block.vector
        def _(eng):
            eng.wait_ge(init, INIT_V)
            dv_off = eng.partition_id() * 64 if one_bcast else None
            for t in range(seq):
                b = t % NBUF
                # layer 0
                eng.wait_ge(pe_g0, t + 1)
                nc.vector.tensor_copy(g0sb.ap(), ps_g0.ap()).then_inc(dv_g0, 1)
                eng.wait_ge(act_g0, t + 1)
                i_t, f_t, g_t, o_t, th_t, t1, t2 = [x.ap() for x in gl[0]]
                nc.vector.tensor_tensor(t1, f_t, cT[0].ap(), ALU.mult)
                nc.vector.tensor_tensor(t2, i_t, g_t, ALU.mult)
                if t >= 1:
                    eng.wait_ge(act_th0, t)      # tanh(c(t-1)) read done
                nc.vector.tensor_tensor(cT[0].ap(), t1, t2,
                                        ALU.add).then_inc(dv_c0, 1)
                eng.wait_ge(act_th0, t + 1)
                if t >= NBUF and exchanges:
                    # sends from buf b (step t-NBUF) drained
                    eng.wait_ge(ls0, LS_PER * (t - NBUF + 1))
                h0slot = (h0T[b].ap()[:, bass.ds(dv_off, 64)] if one_bcast
                          else h0T[b].ap()[:, 0:64])
                nc.vector.tensor_tensor(h0slot, o_t, th_t,
                                        ALU.mult).then_inc(dv_h0, 1)
                # layer 1
                eng.wait_ge(pe_g1, t + 1)
                nc.vector.tensor_copy(g1sb.ap(), ps_g1.ap()).then_inc(dv_g1, 1)
                eng.wait_ge(act_g1, t + 1)
                i_t, f_t, g_t, o_t, th_t, t1, t2 = [x.ap() for x in gl[1]]
                nc.vector.tensor_tensor(t1, f_t, cT[1].ap(), ALU.mult)
                nc.vector.tensor_tensor(t2, i_t, g_t, ALU.mult)
                if t >= 1:
                    eng.wait_ge(act_th1, t)
                nc.vector.tensor_tensor(cT[1].ap(), t1, t2,
                                        ALU.add).then_inc(dv_c1, 1)
                eng.wait_ge(act_th1, t + 1)
                if t >= NBUF and exchanges:
                    eng.wait_ge(ls1, LS_PER * (t - NBUF + 1))
                h1slot = (h1T[b].ap()[:, bass.ds(dv_off, 64)] if one_bcast
                          else h1T[b].ap()[:, 0:64])
                nc.vector.tensor_tensor(h1slot, o_t, th_t,
                                        ALU.mult).then_inc(dv_h1, 1)

        @block.gpsimd
        def _(eng):
            eng.wait_ge(init, INIT_V)
            if one_bcast and exchanges:
                gp_off = eng.partition_id() * 64
                rdests = [None] + [(0, d ^ 2) if d >= 4 else (0, d)
                                   for d in range(1, 8)]

                def stage(t):
                    b = t % NBUF
                    # h0 frames ride SWDGE queue 0, h1 frames queue 1, so the
                    # two sends of a step drain concurrently on the SDMA side
                    for buf, rsem, lsem, qn in ((h0T[b], rs_h0[b], ls0, 0),
                                                (h1T[b], rs_h1[b], ls1, 1)):
                        slot = buf.ap()[:, bass.ds(gp_off, 64)]
                        eng.remote_dma_broadcast(
                            slot, slot, remote_sem=rsem, local_sem=lsem,
                            rdests=rdests, queue_num=qn).then_inc(prep, 1)

                stage(0)
                for t in range(seq):
                    eng.wait_ge(prep, 2 * t + 1)
                    eng.wait_ge(dv_h0, t + 1)
                    eng.trigger_dma(count=1, queue_num=0)
                    eng.wait_ge(prep, 2 * t + 2)
                    eng.wait_ge(dv_h1, t + 1)
                    eng.trigger_dma(count=1, queue_num=1)
                    if t + 1 < seq:
                        stage(t + 1)
            else:
                gp_off = eng.partition_id() * 64 if one_bcast else None
                nprep = 0
                for t in range(seq if exchanges else 0):
                    p = t % NBUF
                    for buf, hsem, rsem, lsem in ((h0T[p], dv_h0, rs_h0[p], ls0),
                                                  (h1T[p], dv_h1, rs_h1[p], ls1)):
                        eng.wait_ge(hsem, t + 1)
                        for d in range(1, 8):
                            rdests2 = [None] * 8
                            rdests2[d] = (0, d ^ 2) if d >= 4 else (0, d)
                            eng.remote_dma_broadcast(
                                buf.ap()[:, 64 * d:64 * (d + 1)],
                                buf.ap()[:, 0:64],
                                remote_sem=rsem, local_sem=lsem,
                                rdests=rdests2).then_inc(prep, 1)
                        nprep += 7
                        eng.wait_ge(prep, nprep)
                        eng.trigger_dma(count=7)

    nc.compile()
    return nc


def _prep_inputs(core, W_ih0, W_hh0, b_ih0, b_hh0, W_ih1, W_hh1, b_ih1, b_hh1,
                 W_fc, b_fc, h0, c0, rotate=False):
    c = core
    rows = np.concatenate([np.arange(g * H + c * HPC, g * H + (c + 1) * HPC)
                           for g in range(4)])
    if rotate:
        hperm = np.concatenate([np.arange((c ^ j) * HPC, ((c ^ j) + 1) * HPC)
                                for j in range(8)])
    else:
        hperm = np.arange(H)
    import ml_dtypes
    f = np.float32
    bf = ml_dtypes.bfloat16
    w0 = np.concatenate([W_ih0[rows].T, W_hh0[rows].T[hperm]], axis=0)
    w1 = np.concatenate([W_ih1[rows].T[hperm], W_hh1[rows].T[hperm]], axis=0)
    wfc = W_fc.T[hperm]
    return {
        "w0": np.ascontiguousarray(w0).astype(bf),
        "w1": np.ascontiguousarray(w1).astype(bf),
        "wfc": np.ascontiguousarray(wfc).astype(bf),
        "b0": np.ascontiguousarray((b_ih0 + b_hh0)[rows].reshape(4, HPC).T, f),
        "b1": np.ascontiguousarray((b_ih1 + b_hh1)[rows].reshape(4, HPC).T, f),
        "bfc": np.ascontiguousarray(b_fc.reshape(4, HPC).T, f),
        "h0i": np.ascontiguousarray(h0[0].T[hperm]).astype(bf),
        "h1i": np.ascontiguousarray(h0[1].T[hperm]).astype(bf),
        "c0i": np.ascontiguousarray(c0[0][:, c * HPC:(c + 1) * HPC].T, f),
        "c1i": np.ascontiguousarray(c0[1][:, c * HPC:(c + 1) * HPC].T, f),
        "yi": np.full((512, 64), SOS_VALUE, bf),
        "ident": np.eye(128, dtype=f),
    }


def run(seq, in_maps, trace=False, trace_kwargs=None):
    from concourse.bass_utils import run_bass_kernel_spmd

    key = int(seq)
    if key not in _CACHE:
        _CACHE[key] = _build(key)
    nc = _CACHE[key]
    kw = {}
    if trace:
        kw = dict(trace=True, trace_cores=[0], **(trace_kwargs or {}))
    return run_bass_kernel_spmd(nc, in_maps, core_ids=list(range(N_CORES)),
                                **kw)


def kernel(encoder_output=None, h0=None, c0=None, W_ih0=None, W_hh0=None,
           b_ih0=None, b_hh0=None, W_ih1=None, W_hh1=None, b_ih1=None,
           b_hh1=None, W_fc=None, b_fc=None, seq_length=256, _trace=False):
    seq = int(seq_length)
    args = (W_ih0, W_hh0, b_ih0, b_hh0, W_ih1, W_hh1, b_ih1, b_hh1, W_fc, b_fc,
            h0, c0)
    args = tuple(np.asarray(a, np.float32) for a in args)
    in_maps = [_prep_inputs(c, *args) for c in range(N_CORES)]
    res = run(seq, in_maps, trace=_trace)
    out = np.asarray(res.results[0]["out"]).astype(np.float32)
    y = out.reshape(seq, 128, 4, 64).transpose(3, 0, 2, 1).reshape(B, seq, OUT)
    if _trace:
        kernel._last_results = res
    return np.ascontiguousarray(y)



# revision 31
# speedup vs baseline: 10.4200x; 1.0839x over previous
"""Trainium2 Bass kernel for a 2-layer LSTM decoder (nn_Decoder).

Strategy: tensor-parallel over the hidden dimension across 8 NeuronCores.
Each core owns a 128-wide slice of H=1024 for both LSTM layers (its 512 of
the 4096 gate rows), and replicates the final fc layer so the autoregressive
input y needs no exchange.  The only cross-core traffic is an allgather of
each layer's hidden-state slice (128x64 fp32 = 32KB) per step, done with
direct SBUF->SBUF remote DMA (no HBM bounce, no ncfw collective floor).

Layouts are feature-on-partition / batch-on-free ("transposed") everywhere.
LSTM gate matmuls run moving-weight orientation: out[batch(64), gates(512)]
= lhsT(activations.T).T @ rhs(W.T) in bf16 (1 cycle/row at N=512), fp32 PSUM
accumulation; gates are then PE-transposed to [hid(128), batch(64)] so
sigmoid/tanh biases ride free on the scalar engine's per-partition bias
operand and the new h slice lands directly in allgather layout.  The fc
matmul runs weight-stationary and produces y.T in PSUM directly.  Cell
state c stays fp32 on its own core slice.

Each exchange is ONE remote_dma_broadcast to the 7 peers (XOR-relative
routing; empirically the D2D lanes (delta bit 2 set) deliver to delta^2 so
those rdests are pre-compensated).  The destination slot is sender-pid *
64 columns via a register access pattern, so every core runs an identical
program.  Descriptor generation is pre-staged one step ahead on the Q7;
only trigger_dma sits on the critical path.

A single 16KB broadcast frame takes ~12us from trigger to send-complete
(HW-measured with a sends-only microbench), so sends must be pipelined,
not serialized: h buffers are 4-deep (NBUF), the sender may run up to 3
sends ahead of the ls (send-complete) semaphore, and h0/h1 frames ride
separate SWDGE queues (0/1) so a step's two sends drain concurrently on
the SDMA engines.  The LSTM's own data-dependency chain (each step needs
all peers' h from the previous step) bounds sender/receiver skew well
below NBUF, so no credit messages are needed.
"""

import sys

sys.path.insert(0, "/opt/trn_rl_repo")

import numpy as np

B = 64
H = 1024
OUT = 512
N_CORES = 8
HPC = H // N_CORES  # 128 hidden units per core
SOS_VALUE = -2.0

_CACHE = {}


def _build(seq, exchanges=True, outdma=True, rswaits=True, one_bcast=True):
    from concourse import bacc, bass, mybir

    dt = mybir.dt
    f32 = dt.float32
    bf16 = dt.bfloat16
    AF = mybir.ActivationFunctionType
    ALU = mybir.AluOpType

    nc = bacc.Bacc("TRN2", target_bir_lowering=False, debug=False,
                   num_devices=N_CORES, num_swdge_queues=2)

    # ---- DRAM I/O (per-core shards prepared on host) ----
    w0_d = nc.dram_tensor("w0", [12 * 128, 512], bf16, kind="ExternalInput")
    w1_d = nc.dram_tensor("w1", [16 * 128, 512], bf16, kind="ExternalInput")
    wfc_d = nc.dram_tensor("wfc", [8 * 128, 512], bf16, kind="ExternalInput")
    b0_d = nc.dram_tensor("b0", [128, 4], f32, kind="ExternalInput")
    b1_d = nc.dram_tensor("b1", [128, 4], f32, kind="ExternalInput")
    bfc_d = nc.dram_tensor("bfc", [128, 4], f32, kind="ExternalInput")
    h0i_d = nc.dram_tensor("h0i", [1024, 64], bf16, kind="ExternalInput")
    h1i_d = nc.dram_tensor("h1i", [1024, 64], bf16, kind="ExternalInput")
    c0i_d = nc.dram_tensor("c0i", [128, 64], f32, kind="ExternalInput")
    c1i_d = nc.dram_tensor("c1i", [128, 64], f32, kind="ExternalInput")
    yi_d = nc.dram_tensor("yi", [512, 64], bf16, kind="ExternalInput")
    id_d = nc.dram_tensor("ident", [128, 128], f32, kind="ExternalInput")
    out_d = nc.dram_tensor("out", [seq, 128, 256], bf16, kind="ExternalOutput")

    # ---- SBUF ----
    w0 = nc.alloc_sbuf_tensor("w0s", [128, 12 * 512], bf16)
    w1 = nc.alloc_sbuf_tensor("w1s", [128, 16 * 512], bf16)
    wfc = nc.alloc_sbuf_tensor("wfcs", [128, 8 * 512], bf16)
    b0 = nc.alloc_sbuf_tensor("b0s", [128, 4], f32)
    b1 = nc.alloc_sbuf_tensor("b1s", [128, 4], f32)
    bfc = nc.alloc_sbuf_tensor("bfcs", [128, 4], f32)
    ident = nc.alloc_sbuf_tensor("idents", [128, 128], f32)
    NBUF = 4  # h-exchange depth: unchokes the SWDGE send chain
    h0T = [nc.alloc_sbuf_tensor(f"h0T{p}", [128, 512], bf16)
           for p in range(NBUF)]
    h1T = [nc.alloc_sbuf_tensor(f"h1T{p}", [128, 512], bf16)
           for p in range(NBUF)]
    xT = [nc.alloc_sbuf_tensor(f"xT{p}", [128, 256], bf16) for p in range(2)]
    cT = [nc.alloc_sbuf_tensor(f"cT{l}", [128, 64], f32) for l in range(2)]
    g0sb = nc.alloc_sbuf_tensor("g0sb", [64, 512], f32)
    g1sb = nc.alloc_sbuf_tensor("g1sb", [64, 512], f32)
    gl = [[nc.alloc_sbuf_tensor(f"g{l}_{n}", [128, 64], f32)
           for n in ("i", "f", "g", "o", "th", "t1", "t2")] for l in range(2)]

    # ---- PSUM ----
    ps_g0 = nc.alloc_psum_tensor("ps_g0", [64, 512], f32)
    ps_g1 = nc.alloc_psum_tensor("ps_g1", [64, 512], f32)
    ps_t0 = nc.alloc_psum_tensor("ps_t0", [128, 256], f32)
    ps_t1 = nc.alloc_psum_tensor("ps_t1", [128, 256], f32)
    ps_ty = nc.alloc_psum_tensor("ps_ty", [128, 256], f32)
    # dead bank for PE-warmth junk matmuls (never read by any engine)
    ps_junk = nc.alloc_psum_tensor("ps_junk", [64, 512], f32)

    # ---- semaphores ----
    S = lambda n: nc.alloc_semaphore(n)
    init = S("init")
    pe_g0, pe_t0 = S("pe_g0"), S("pe_t0")
    pe_g1, pe_t1 = S("pe_g1"), S("pe_t1")
    pe_ty = S("pe_ty")
    act_g0, act_th0 = S("act_g0"), S("act_th0")
    act_g1, act_th1 = S("act_g1"), S("act_th1")
    act_y = S("act_y")
    dv_g0, dv_g1 = S("dv_g0"), S("dv_g1")
    dv_c0, dv_c1 = S("dv_c0"), S("dv_c1")
    dv_h0, dv_h1 = S("dv_h0"), S("dv_h1")
    prep = S("prep")
    ls0, ls1 = S("ls0"), S("ls1")
    rs_h0 = [S(f"rs_h0_{p}") for p in range(NBUF)]
    rs_h1 = [S(f"rs_h1_{p}") for p in range(NBUF)]
    dsem = S("dsem")

    N_INIT_DMA = 12
    INIT_V = 16 * N_INIT_DMA

    LS_PER = 16 if one_bcast else 112
    NWARM = 10  # junk matmuls per exchange window (~2.2us of PE warmth)

    def fills(t):
        # number of exchange rounds into buffer t%NBUF after step t's exchange
        return t // NBUF + 1

    def wtile(sb, k):
        return sb.ap()[:, 512 * k:512 * (k + 1)]

    def htile(sb, k):
        return sb.ap()[:, 64 * k:64 * (k + 1)]

    with nc.Block() as block:

        @block.sync
        def _(eng):
            eng.dma_start(
                w0.ap().rearrange("p (t n) -> p t n", t=12),
                w0_d.ap().rearrange("(t p) n -> p t n", p=128)).then_inc(init, 16)
            eng.dma_start(
                w1.ap().rearrange("p (t n) -> p t n", t=16),
                w1_d.ap().rearrange("(t p) n -> p t n", p=128)).then_inc(init, 16)
            eng.dma_start(
                wfc.ap().rearrange("p (t n) -> p t n", t=8),
                wfc_d.ap().rearrange("(t p) n -> p t n", p=128)).then_inc(init, 16)
            eng.dma_start(b0.ap(), b0_d.ap()).then_inc(init, 16)
            eng.dma_start(b1.ap(), b1_d.ap()).then_inc(init, 16)
            eng.dma_start(bfc.ap(), bfc_d.ap()).then_inc(init, 16)
            eng.dma_start(
                h0T[NBUF - 1].ap().rearrange("p (t n) -> p t n", t=8),
                h0i_d.ap().rearrange("(t p) n -> p t n", p=128)).then_inc(init, 16)
            eng.dma_start(
                h1T[NBUF - 1].ap().rearrange("p (t n) -> p t n", t=8),
                h1i_d.ap().rearrange("(t p) n -> p t n", p=128)).then_inc(init, 16)
            eng.dma_start(cT[0].ap(), c0i_d.ap()).then_inc(init, 16)
            eng.dma_start(cT[1].ap(), c1i_d.ap()).then_inc(init, 16)
            eng.dma_start(
                xT[1].ap().rearrange("p (t n) -> p t n", t=4),
                yi_d.ap().rearrange("(t p) n -> p t n", p=128)).then_inc(init, 16)
            eng.dma_start(ident.ap(), id_d.ap()).then_inc(init, 16)
            for t in range(seq if outdma else 0):
                eng.wait_ge(act_y, t + 1)
                eng.dma_start(
                    out_d.ap()[t], xT[t % 2].ap()).then_inc(dsem, 16)

        @block.tensor
        def _(eng):
            eng.wait_ge(init, INIT_V)
            # prologue: L0 hh-part for t=0 (reads initial h0 in buf NBUF-1)
            for k in range(8):
                nc.tensor.matmul(ps_g0.ap(), htile(h0T[NBUF - 1], k),
                                 wtile(w0, 4 + k),
                                 start=(k == 0), stop=False)
            for t in range(seq):
                p, q = t % 2, (t + 1) % 2
                b, pb = t % NBUF, (t + NBUF - 1) % NBUF
                # ---- layer 0 gates: close the group with the x-part ----
                if t >= 1:
                    eng.wait_ge(act_y, t)        # x = y(t-1) ready in xT[q]
                for k in range(4):
                    mm = nc.tensor.matmul(ps_g0.ap(), htile(xT[q], k),
                                          wtile(w0, k),
                                          start=False, stop=(k == 3))
                mm.then_inc(pe_g0, 1)
                # early L1-hh matmuls overlap the DVE gate copy
                if t >= 1:
                    eng.wait_ge(dv_g1, t)
                    eng.wait_ge(dv_h1, t)
                    if exchanges and rswaits:
                        eng.wait_ge(rs_h1[pb], 14 * fills(t - 1))
                for k in range(3):
                    nc.tensor.matmul(ps_g1.ap(), htile(h1T[pb], k),
                                     wtile(w1, 8 + k),
                                     start=(k == 0), stop=False)
                # ---- transpose gates0 to [128, 4*64] ----
                eng.wait_ge(dv_g0, t + 1)        # g0sb written by DVE
                if t >= 1:
                    eng.wait_ge(act_g0, t)       # ps_t0 consumed by ACT
                for j in range(4):
                    mm = nc.tensor.matmul(ps_t0.ap()[:, 64 * j:64 * (j + 1)],
                                          g0sb.ap()[:, 128 * j:128 * (j + 1)],
                                          ident.ap()[:64, :64],
                                          is_transpose=True, start=True,
                                          stop=True)
                mm.then_inc(pe_t0, 1)
                # ---- layer 1 gates: finish hh-part, then fresh-h0 ih-part ----
                for k in range(3, 8):
                    nc.tensor.matmul(ps_g1.ap(), htile(h1T[pb], k),
                                     wtile(w1, 8 + k),
                                     start=False, stop=False)
                # HAM warmth: the h0-exchange stall below is ~4-9us of PE
                # idle, longer than the 3.4us HAM MID window, so the PE
                # would re-throttle to 1.2GHz every step.  Junk matmuls on
                # stale data into a dead PSUM bank keep the activity window
                # busy; they execute inside the stall so they are free.
                for k in range(NWARM):
                    nc.tensor.matmul(ps_junk.ap(), htile(h0T[pb], k % 8),
                                     wtile(w1, k % 8), start=True, stop=True)
                eng.wait_ge(dv_h0, t + 1)        # own h0(t) slice
                if exchanges and rswaits:
                    eng.wait_ge(rs_h0[b], 14 * fills(t))  # peers' h0(t)
                for k in range(8):
                    mm = nc.tensor.matmul(ps_g1.ap(), htile(h0T[b], k),
                                          wtile(w1, k),
                                          start=False, stop=(k == 7))
                mm.then_inc(pe_g1, 1)
                # ---- transpose gates1 ----
                eng.wait_ge(dv_g1, t + 1)
                if t >= 1:
                    eng.wait_ge(act_g1, t)
                for j in range(4):
                    mm = nc.tensor.matmul(ps_t1.ap()[:, 64 * j:64 * (j + 1)],
                                          g1sb.ap()[:, 128 * j:128 * (j + 1)],
                                          ident.ap()[:64, :64],
                                          is_transpose=True, start=True,
                                          stop=True)
                mm.then_inc(pe_t1, 1)
                # ---- L0 hh-part for step t+1 (fills the h1-exchange window;
                # h0(t) already gathered, ps_g0 drained once dv_g0 hits t+1) ----
                if t + 1 < seq:
                    eng.wait_ge(dv_g0, t + 1)
                    eng.wait_ge(dv_h0, t + 1)
                    if exchanges and rswaits:
                        eng.wait_ge(rs_h0[b], 14 * fills(t))
                    for k in range(8):
                        nc.tensor.matmul(ps_g0.ap(), htile(h0T[b], k),
                                         wtile(w0, 4 + k),
                                         start=(k == 0), stop=False)
                # HAM warmth through the h1-exchange stall (see above)
                for k in range(NWARM):
                    nc.tensor.matmul(ps_junk.ap(), htile(h1T[pb], k % 8),
                                     wtile(w1, 8 + k % 8), start=True,
                                     stop=True)
                # ---- fc (replicated, weight-stationary): y.T into ps_ty ----
                eng.wait_ge(dv_h1, t + 1)
                if exchanges and rswaits:
                    eng.wait_ge(rs_h1[b], 14 * fills(t))
                if t >= 1:
                    eng.wait_ge(act_y, t)        # ps_ty consumed by ACT(t-1)
                for m in range(4):
                    for k in range(8):
                        mm = nc.tensor.matmul(
                            ps_ty.ap()[:, 64 * m:64 * (m + 1)],
                            wfc.ap()[:, 512 * k + 128 * m:512 * k + 128 * (m + 1)],
                            htile(h1T[b], k),
                            start=(k == 0), stop=(k == 7))
                mm.then_inc(pe_ty, 1)

        @block.scalar
        def _(eng):
            eng.wait_ge(init, INIT_V)
            for t in range(seq):
                p = t % 2
                for l, (ps_t, gsem, thsem, csem, bias) in enumerate(
                        ((ps_t0, act_g0, act_th0, dv_c0, b0),
                         (ps_t1, act_g1, act_th1, dv_c1, b1))):
                    eng.wait_ge((pe_t0, pe_t1)[l], t + 1)
                    i_t, f_t, g_t, o_t, th_t = [x.ap() for x in gl[l][:5]]
                    src = ps_t.ap()
                    a = nc.scalar.activation(i_t, src[:, 0:64], AF.Sigmoid,
                                             bias=bias.ap()[:, 0:1])
                    a = nc.scalar.activation(f_t, src[:, 64:128], AF.Sigmoid,
                                             bias=bias.ap()[:, 1:2])
                    a = nc.scalar.activation(g_t, src[:, 128:192], AF.Tanh,
                                             bias=bias.ap()[:, 2:3])
                    a = nc.scalar.activation(o_t, src[:, 192:256], AF.Sigmoid,
                                             bias=bias.ap()[:, 3:4])
                    a.then_inc(gsem, 1)
                    eng.wait_ge(csem, t + 1)
                    nc.scalar.activation(th_t, cT[l].ap(),
                                         AF.Tanh).then_inc(thsem, 1)
                # fc relu -> xT[p]
                eng.wait_ge(pe_ty, t + 1)
                if t >= 2 and outdma:
                    eng.wait_ge(dsem, 16 * (t - 1))   # out-DMA(t-2) done
                for j in range(4):
                    a = nc.scalar.activation(xT[p].ap()[:, 64 * j:64 * (j + 1)],
                                             ps_ty.ap()[:, 64 * j:64 * (j + 1)],
                                             AF.Relu, bias=bfc.ap()[:, j:j + 1])
                a.then_inc(act_y, 1)

        @block.vector
        def _(eng):
            eng.wait_ge(init, INIT_V)
            dv_off = eng.partition_id() * 64 if one_bcast else None
            for t in range(seq):
                b = t % NBUF
                # layer 0
                eng.wait_ge(pe_g0, t + 1)
                nc.vector.tensor_copy(g0sb.ap(), ps_g0.ap()).then_inc(dv_g0, 1)
                eng.wait_ge(act_g0, t + 1)
                i_t, f_t, g_t, o_t, th_t, t1, t2 = [x.ap() for x in gl[0]]
                nc.vector.tensor_tensor(t1, f_t, cT[0].ap(), ALU.mult)
                nc.vector.tensor_tensor(t2, i_t, g_t, ALU.mult)
                if t >= 1:
                    eng.wait_ge(act_th0, t)      # tanh(c(t-1)) read done
                nc.vector.tensor_tensor(cT[0].ap(), t1, t2,
                                        ALU.add).then_inc(dv_c0, 1)
                eng.wait_ge(act_th0, t + 1)
                if t >= NBUF and exchanges:
                    # sends from buf b (step t-NBUF) drained
                    eng.wait_ge(ls0, LS_PER * (t - NBUF + 1))
                h0slot = (h0T[b].ap()[:, bass.ds(dv_off, 64)] if one_bcast
                          else h0T[b].ap()[:, 0:64])
                nc.vector.tensor_tensor(h0slot, o_t, th_t,
                                        ALU.mult).then_inc(dv_h0, 1)
                # layer 1
                eng.wait_ge(pe_g1, t + 1)
                nc.vector.tensor_copy(g1sb.ap(), ps_g1.ap()).then_inc(dv_g1, 1)
                eng.wait_ge(act_g1, t + 1)
                i_t, f_t, g_t, o_t, th_t, t1, t2 = [x.ap() for x in gl[1]]
                nc.vector.tensor_tensor(t1, f_t, cT[1].ap(), ALU.mult)
                nc.vector.tensor_tensor(t2, i_t, g_t, ALU.mult)
                if t >= 1:
                    eng.wait_ge(act_th1, t)
                nc.vector.tensor_tensor(cT[1].ap(), t1, t2,
                                        ALU.add).then_inc(dv_c1, 1)
                eng.wait_ge(act_th1, t + 1)
                if t >= NBUF and exchanges:
                    eng.wait_ge(ls1, LS_PER * (t - NBUF + 1))
                h1slot = (h1T[b].ap()[:, bass.ds(dv_off, 64)] if one_bcast
                          else h1T[b].ap()[:, 0:64])
                nc.vector.tensor_tensor(h1slot, o_t, th_t,
                                        ALU.mult).then_inc(dv_h1, 1)

        @block.gpsimd
        def _(eng):
            eng.wait_ge(init, INIT_V)
            if one_bcast and exchanges:
                gp_off = eng.partition_id() * 64
                rdests = [None] + [(0, d ^ 2) if d >= 4 else (0, d)
                                   for d in range(1, 8)]

                def stage(t):
                    b = t % NBUF
                    # h0 frames ride SWDGE queue 0, h1 frames queue 1, so the
                    # two sends of a step drain concurrently on the SDMA side
                    for buf, rsem, lsem, qn in ((h0T[b], rs_h0[b], ls0, 0),
                                                (h1T[b], rs_h1[b], ls1, 1)):
                        slot = buf.ap()[:, bass.ds(gp_off, 64)]
                        eng.remote_dma_broadcast(
                            slot, slot, remote_sem=rsem, local_sem=lsem,
                            rdests=rdests, queue_num=qn).then_inc(prep, 1)

                stage(0)
                for t in range(seq):
                    eng.wait_ge(prep, 2 * t + 1)
                    eng.wait_ge(dv_h0, t + 1)
                    eng.trigger_dma(count=1, queue_num=0)
                    eng.wait_ge(prep, 2 * t + 2)
                    eng.wait_ge(dv_h1, t + 1)
                    eng.trigger_dma(count=1, queue_num=1)
                    if t + 1 < seq:
                        stage(t + 1)
            else:
                gp_off = eng.partition_id() * 64 if one_bcast else None
                nprep = 0
                for t in range(seq if exchanges else 0):
                    p = t % NBUF
                    for buf, hsem, rsem, lsem in ((h0T[p], dv_h0, rs_h0[p], ls0),
                                                  (h1T[p], dv_h1, rs_h1[p], ls1)):
                        eng.wait_ge(hsem, t + 1)
                        for d in range(1, 8):
                            rdests2 = [None] * 8
                            rdests2[d] = (0, d ^ 2) if d >= 4 else (0, d)
                            eng.remote_dma_broadcast(
                                buf.ap()[:, 64 * d:64 * (d + 1)],
                                buf.ap()[:, 0:64],
                                remote_sem=rsem, local_sem=lsem,
                                rdests=rdests2).then_inc(prep, 1)
                        nprep += 7
                        eng.wait_ge(prep, nprep)
                        eng.trigger_dma(count=7)

    nc.compile()
    return nc


def _prep_inputs(core, W_ih0, W_hh0, b_ih0, b_hh0, W_ih1, W_hh1, b_ih1, b_hh1,
                 W_fc, b_fc, h0, c0, rotate=False):
    c = core
    rows = np.concatenate([np.arange(g * H + c * HPC, g * H + (c + 1) * HPC)
                           for g in range(4)])
    if rotate:
        hperm = np.concatenate([np.arange((c ^ j) * HPC, ((c ^ j) + 1) * HPC)
                                for j in range(8)])
    else:
        hperm = np.arange(H)
    import ml_dtypes
    f = np.float32
    bf = ml_dtypes.bfloat16
    w0 = np.concatenate([W_ih0[rows].T, W_hh0[rows].T[hperm]], axis=0)
    w1 = np.concatenate([W_ih1[rows].T[hperm], W_hh1[rows].T[hperm]], axis=0)
    wfc = W_fc.T[hperm]
    return {
        "w0": np.ascontiguousarray(w0).astype(bf),
        "w1": np.ascontiguousarray(w1).astype(bf),
        "wfc": np.ascontiguousarray(wfc).astype(bf),
        "b0": np.ascontiguousarray((b_ih0 + b_hh0)[rows].reshape(4, HPC).T, f),
        "b1": np.ascontiguousarray((b_ih1 + b_hh1)[rows].reshape(4, HPC).T, f),
        "bfc": np.ascontiguousarray(b_fc.reshape(4, HPC).T, f),
        "h0i": np.ascontiguousarray(h0[0].T[hperm]).astype(bf),
        "h1i": np.ascontiguousarray(h0[1].T[hperm]).astype(bf),
        "c0i": np.ascontiguousarray(c0[0][:, c * HPC:(c + 1) * HPC].T, f),
        "c1i": np.ascontiguousarray(c0[1][:, c * HPC:(c + 1) * HPC].T, f),
        "yi": np.full((512, 64), SOS_VALUE, bf),
        "ident": np.eye(128, dtype=f),
    }


def run(seq, in_maps, trace=False, trace_kwargs=None):
    from concourse.bass_utils import run_bass_kernel_spmd

    key = int(seq)
    if key not in _CACHE:
        _CACHE[key] = _build(key)
    nc = _CACHE[key]
    kw = {}
    if trace:
        kw = dict(trace=True, trace_cores=[0], **(trace_kwargs or {}))
    return run_bass_kernel_spmd(nc, in_maps, core_ids=list(range(N_CORES)),
                                **kw)


def kernel(encoder_output=None, h0=None, c0=None, W_ih0=None, W_hh0=None,
           b_ih0=None, b_hh0=None, W_ih1=None, W_hh1=None, b_ih1=None,
           b_hh1=None, W_fc=None, b_fc=None, seq_length=256, _trace=False):
    seq = int(seq_length)
    args = (W_ih0, W_hh0, b_ih0, b_hh0, W_ih1, W_hh1, b_ih1, b_hh1, W_fc, b_fc,
            h0, c0)
    args = tuple(np.asarray(a, np.float32) for a in args)
    in_maps = [_prep_inputs(c, *args) for c in range(N_CORES)]
    res = run(seq, in_maps, trace=_trace)
    out = np.asarray(res.results[0]["out"]).astype(np.float32)
    y = out.reshape(seq, 128, 4, 64).transpose(3, 0, 2, 1).reshape(B, seq, OUT)
    if _trace:
        kernel._last_results = res
    return np.ascontiguousarray(y)

